# revision 4
# baseline (speedup 1.0000x reference)
"""Trainium2 Bass kernel for Conv2D_DT (distance-transform conv).

d(n,o,h,w) = || patch(n,:,h,w) - W[o,:] ||_2  with 3x3/pad1 im2col patches.

Strategy (8 NeuronCores, data-parallel over batch):
  - 4 images per core, processed as 2 pairs: image A on SBUF partitions
    0-63, image B on partitions 64-127 (channels = partition dim).
  - d2 = ||p||^2 + ||w||^2 - 2 p.w  accumulated fully in PSUM:
      * 9 shifted matmuls (taps) with lhsT = -2*W_tap, bf16 [K=64/image]
      * 1 matmul with lhsT = ones (bf16) over b = 3x3 box sum of x^2,
        which is the whole ||p||^2 term (channel sum via the contraction)
  - The two images' K=64 matmuls land on PE row-groups (0,0)/(64,0) and
    run concurrently -> full 128-row array utilization (~78 TF/s).
  - Preprocessing in bf16: sq = x*x on GpSimd (Scalar for the first
    group, which is on the critical path), 4 shifted adds on DVE for the
    separable 3x3 box sum.  Only the bf16 input is DMA'd; output is
    written as fp16 and upcast on host.
  - Startup: PE-warmup matmuls over memset scratch pay the DVFS ramp
    during the DMA-fill window; the slot-0 tap weights are a separate
    small DMA so the first real matmul is gated on ~300KB; image-B first
    group goes on the Scalar HWDGE queue in parallel with SP.
  - epilogue: ScalarE  out = Sqrt(psum + w2[o]) -> fp16, one output DMA
    per 16-row group.  (d2 >= ~200 for this data, Sqrt is safe.)
"""

import sys

_REPO = "/opt/trn_rl_repo"
if _REPO not in sys.path:
    sys.path.insert(0, _REPO)

import ml_dtypes
import numpy as np

import concourse.bass as bass  # noqa: F401
import concourse.mybir as mybir
import concourse.tile as tile
from concourse import bacc
from concourse.bass_utils import run_bass_kernel_spmd

# Problem geometry (hardcoded per harness contract).
N, C, H, W_DIM, O = 32, 64, 56, 56, 128
NCORES = 8
NL = N // NCORES  # images per core
NPAIR = NL // 2  # image pairs per core
HP = WP = 58  # zero-padded spatial dims
RCH = 8  # output rows per PSUM chunk
NCH = H // RCH  # 7 chunks per image
NXTAP = 9
DELAY = 1  # chunks between taps and box-matmul/epilogue
NWARM = 5  # PE-warmup matmuls to pay the DVFS ramp during DMA fill

F32 = mybir.dt.float32
F16 = mybir.dt.float16
BF16 = mybir.dt.bfloat16

# (r0, R, chunks): padded-row window [r0, r0+R) covering output chunks
GROUPS = ((0, 18, (0, 1)), (16, 18, (2, 3)), (32, 18, (4, 5)), (48, 10, (6,)))

_PROGRAM = None


def _build_program():
    nc = bacc.Bacc(
        "TRN2",
        target_bir_lowering=False,
        debug=False,
        enable_asserts=False,
        num_devices=NCORES,
    )
    xsb = nc.dram_tensor("xsb", [NL, C, HP, WP], BF16, kind="ExternalInput")
    lwb = nc.dram_tensor("lwb", [128, NXTAP, 128], BF16, kind="ExternalInput")
    lwo = nc.dram_tensor("lwo", [128, 128], BF16, kind="ExternalInput")
    w2 = nc.dram_tensor("w2", [128, 1], F32, kind="ExternalInput")
    out = nc.dram_tensor("out", [NL, O, H, W_DIM], F16, kind="ExternalOutput")

    with tile.TileContext(nc) as tc:
        with (
            tc.tile_pool(name="const", bufs=1) as cpool,
            tc.tile_pool(name="imgs", bufs=2) as ipool,
            tc.tile_pool(name="outs", bufs=4) as opool,
            tc.tile_pool(name="psum", bufs=8, space="PSUM") as ppool,
        ):
            # PE warmup: memset scratch, then NWARM dummy matmuls so the
            # tensor engine's DVFS ramp happens during the DMA fill.
            wlhs = cpool.tile([128, 128], BF16)
            nc.gpsimd.memset(wlhs[:], 0.0)
            wrhs = cpool.tile([128, RCH, W_DIM], BF16)
            nc.gpsimd.memset(wrhs[:], 0.0)
            wps = ppool.tile([128, RCH, W_DIM], F32, tag="ps", name="wps")
            for i in range(NWARM):
                nc.tensor.matmul(
                    wps[:],
                    wlhs[:],
                    wrhs[:],
                    start=(i == 0),
                    stop=(i == NWARM - 1),
                )

            # Weights: slot-0 taps split out so the first matmul is gated
            # on a small load.  Issued on the Scalar HWDGE queue.
            lwb0 = cpool.tile([128, 1, 128], BF16)
            nc.scalar.dma_start(out=lwb0[:], in_=lwb[:, 0:1, :])
            # First pair / first group image-B load in parallel with SP.
            r0_0, R_0, _ = GROUPS[0]
            xbh0 = ipool.tile([128, R_0, WP], BF16, tag="xbh0", name="xbh0")
            nc.sync.dma_start(out=xbh0[0:64, :, :], in_=xsb[0, :, r0_0 : r0_0 + R_0, :])
            nc.scalar.dma_start(
                out=xbh0[64:128, :, :], in_=xsb[1, :, r0_0 : r0_0 + R_0, :]
            )
            lwbr = cpool.tile([128, NXTAP - 1, 128], BF16)
            nc.scalar.dma_start(out=lwbr[:], in_=lwb[:, 1:NXTAP, :])
            lwot = cpool.tile([128, 128], BF16)
            nc.scalar.dma_start(out=lwot[:], in_=lwo[:, :])
            w2t = cpool.tile([128, 1], F32)
            nc.scalar.dma_start(out=w2t[:], in_=w2[:, :])

            def tap_w(slot):
                return lwb0[:, 0, :] if slot == 0 else lwbr[:, slot - 1, :]

            # Remaining input DMAs up-front on SP, in consumption order.
            xtiles = {(0, 0): xbh0}
            for p in range(NPAIR):
                na, nb = 2 * p, 2 * p + 1
                for gi, (r0, R, _chs) in enumerate(GROUPS):
                    if (p, gi) in xtiles:
                        continue
                    xbh = ipool.tile(
                        [128, R, WP], BF16, tag=f"xbh{gi}", name=f"xbh_{p}_{gi}"
                    )
                    nc.sync.dma_start(
                        out=xbh[0:64, :, :], in_=xsb[na, :, r0 : r0 + R, :]
                    )
                    nc.sync.dma_start(
                        out=xbh[64:128, :, :], in_=xsb[nb, :, r0 : r0 + R, :]
                    )
                    xtiles[(p, gi)] = xbh

            ots = {}

            def finish(item):
                ch, na, nb, psa, psb, bh, r0, gi, last_in_group = item
                lb = ch * RCH - r0
                for half, ps in ((slice(0, 64), psa), (slice(64, 128), psb)):
                    nc.tensor.matmul(
                        ps[:],
                        lwot[half, :],
                        bh[half, lb : lb + RCH, :],
                        start=False,
                        stop=True,
                    )
                grows = 8 * len(GROUPS[gi][2])
                row = (ch * RCH - r0) % grows
                if row == 0:
                    for n_img in (na, nb):
                        ots[n_img] = opool.tile(
                            [128, grows, W_DIM], F16, tag="ot", name=f"ot{n_img}_{gi}"
                        )
                for ps, n_img in ((psa, na), (psb, nb)):
                    ot = ots[n_img]
                    nc.scalar.activation(
                        out=ot[:, row : row + RCH, :],
                        in_=ps[:],
                        func=mybir.ActivationFunctionType.Sqrt,
                        bias=w2t[:],
                        scale=1.0,
                    )
                    if last_in_group:
                        nc.sync.dma_start(
                            out=out[n_img, :, r0 : r0 + grows, :],
                            in_=ot[:, :, :],
                        )

            pending = []
            for p in range(NPAIR):
                na, nb = 2 * p, 2 * p + 1
                for gi, (r0, R, chs) in enumerate(GROUPS):
                    xbh = xtiles[(p, gi)]
                    # sq = x*x: Scalar for the critical first group, else GpSimd
                    sq = ipool.tile([128, R, WP], BF16, tag=f"sq{gi}", name=f"sq{gi}")
                    if p == 0 and gi == 0:
                        nc.scalar.activation(
                            out=sq[:],
                            in_=xbh[:],
                            func=mybir.ActivationFunctionType.Square,
                        )
                    else:
                        nc.gpsimd.tensor_mul(sq[:], xbh[:], xbh[:])
                    # separable 3x3 box sum on DVE, all bf16
                    uh = ipool.tile([128, R, W_DIM], BF16, tag=f"uh{gi}", name=f"uh{gi}")
                    nc.vector.tensor_add(uh[:], sq[:, :, 0:56], sq[:, :, 1:57])
                    tth = ipool.tile(
                        [128, R, W_DIM], BF16, tag=f"tth{gi}", name=f"tth{gi}"
                    )
                    nc.vector.tensor_add(tth[:], uh[:], sq[:, :, 2:58])
                    vh = ipool.tile(
                        [128, R - 2, W_DIM], BF16, tag=f"vh{gi}", name=f"vh{gi}"
                    )
                    nc.vector.tensor_add(
                        vh[:], tth[:, 0 : R - 2, :], tth[:, 1 : R - 1, :]
                    )
                    bh = ipool.tile(
                        [128, R - 2, W_DIM], BF16, tag=f"bh{gi}", name=f"bh{gi}"
                    )
                    nc.vector.tensor_add(bh[:], vh[:], tth[:, 2:R, :])

                    for ci, ch in enumerate(chs):
                        lh = ch * RCH - r0  # chunk's first row, local to group
                        psa = ppool.tile([128, RCH, W_DIM], F32, tag="ps", name="psa")
                        psb = ppool.tile([128, RCH, W_DIM], F32, tag="ps", name="psb")
                        for slot in range(NXTAP):
                            kh, kw = divmod(slot, 3)
                            rhs = xbh[:, lh + kh : lh + kh + RCH, kw : kw + 56]
                            st = slot == 0
                            lw = tap_w(slot)
                            nc.tensor.matmul(
                                psa[:],
                                lw[0:64],
                                rhs[0:64],
                                start=st,
                                stop=False,
                            )
                            nc.tensor.matmul(
                                psb[:],
                                lw[64:128],
                                rhs[64:128],
                                start=st,
                                stop=False,
                            )
                        pending.append(
                            (ch, na, nb, psa, psb, bh, r0, gi, ci == len(chs) - 1)
                        )
                        if len(pending) > DELAY:
                            finish(pending.pop(0))
            for item in pending:
                finish(item)
    nc.compile()
    return nc


def _host_weights(W):
    """bf16 x-tap lhsT [128, 9, 128] (dup on both halves), bf16 ones, w2."""
    W = np.asarray(W, np.float32)
    lhs = np.zeros((128, NXTAP, 128), np.float32)
    cidx = np.arange(C)
    for kh in range(3):
        for kw in range(3):
            slot = kh * 3 + kw
            blk = (-2.0 * W[:, cidx * 9 + kh * 3 + kw]).T  # [C, O]
            lhs[0:64, slot, :] = blk
            lhs[64:128, slot, :] = blk
    lwo = np.ones((128, 128), np.float32)
    w2 = (W * W).sum(axis=1).astype(np.float32).reshape(128, 1)
    return (
        lhs.astype(ml_dtypes.bfloat16),
        lwo.astype(ml_dtypes.bfloat16),
        w2,
    )


def get_program():
    global _PROGRAM
    if _PROGRAM is None:
        _PROGRAM = _build_program()
    return _PROGRAM


def make_in_maps(x, W):
    x = np.asarray(x, np.float32)
    xpad = np.zeros((N, C, HP, WP), np.float32)
    xpad[:, :, 1 : H + 1, 1 : W_DIM + 1] = x
    xpadb = np.ascontiguousarray(xpad.astype(ml_dtypes.bfloat16))
    lwb, lwo, w2 = _host_weights(W)
    return [
        {
            "xsb": xpadb[i * NL : (i + 1) * NL],
            "lwb": lwb,
            "lwo": lwo,
            "w2": w2,
        }
        for i in range(NCORES)
    ]


def kernel(x, W):
    nc = get_program()
    in_maps = make_in_maps(x, W)
    res = run_bass_kernel_spmd(nc, in_maps, list(range(NCORES)))
    outs = [res.results[i]["out"] for i in range(NCORES)]
    return np.concatenate(outs, axis=0).astype(np.float32)


# revision 5
# speedup vs baseline: 1.2509x; 1.2509x over previous
"""Trainium2 Bass kernel for Conv2D_DT (distance-transform conv).

d(n,o,h,w) = || patch(n,:,h,w) - W[o,:] ||_2  with 3x3/pad1 im2col patches.

Strategy (8 NeuronCores, data-parallel over batch):
  - 4 images per core, processed as 2 pairs: image A on SBUF partitions
    0-63, image B on partitions 64-127 (channels = partition dim).
  - d2 = ||p||^2 + ||w||^2 - 2 p.w  accumulated fully in PSUM:
      * 9 shifted matmuls (taps) with lhsT = -2*W_tap, bf16 [K=64/image]
      * 1 matmul with lhsT = ones (bf16) over b = 3x3 box sum of x^2,
        which is the whole ||p||^2 term (channel sum via the contraction)
  - The two images' K=64 matmuls land on PE row-groups (0,0)/(64,0) and
    run concurrently -> full 128-row array utilization (~78 TF/s).
  - All preprocessing in bf16 on DVE: sq = x*x (tensor_mul), 4 shifted
    adds for the separable 3x3 box sum.  Only the bf16 input is DMA'd;
    output is written as fp16 and upcast on host.
  - Startup: PE-warmup matmuls over memset scratch pay the DVFS ramp
    during the DMA-fill window; the slot-0 tap weights are a separate
    small DMA so the first real matmul is gated on a small load.  Both
    images of a group load with one DMA (consecutive in DRAM).
  - epilogue: ScalarE  out = Sqrt(psum + w2[o]) -> fp16, one output DMA
    per row-group.  (d2 >= ~200 for this data, Sqrt is safe.)
"""

import sys

_REPO = "/opt/trn_rl_repo"
if _REPO not in sys.path:
    sys.path.insert(0, _REPO)

import ml_dtypes
import numpy as np

import concourse.bass as bass  # noqa: F401
import concourse.mybir as mybir
import concourse.tile as tile
from concourse import bacc
from concourse.bass_utils import run_bass_kernel_spmd

# Problem geometry (hardcoded per harness contract).
N, C, H, W_DIM, O = 32, 64, 56, 56, 128
NCORES = 8
NL = N // NCORES  # images per core
NPAIR = NL // 2  # image pairs per core
HP = WP = 58  # zero-padded spatial dims
RCH = 8  # output rows per PSUM chunk
NCH = H // RCH  # 7 chunks per image
NXTAP = 9
DELAY = 1  # chunks between taps and box-matmul/epilogue
NWARM = 8  # PE-warmup matmuls to pay the DVFS ramp during DMA fill

F32 = mybir.dt.float32
F16 = mybir.dt.float16
BF16 = mybir.dt.bfloat16

# (r0, R, chunks): padded-row window [r0, r0+R) covering output chunks
GROUPS = ((0, 10, (0,)), (8, 18, (1, 2)), (24, 18, (3, 4)), (40, 18, (5, 6)))

_PROGRAM = None


def _build_program():
    nc = bacc.Bacc(
        "TRN2",
        target_bir_lowering=False,
        debug=False,
        enable_asserts=False,
        num_devices=NCORES,
    )
    xsb = nc.dram_tensor("xsb", [NL, C, HP, WP], BF16, kind="ExternalInput")
    lwb = nc.dram_tensor("lwb", [128, NXTAP, 128], BF16, kind="ExternalInput")
    lwo = nc.dram_tensor("lwo", [128, 128], BF16, kind="ExternalInput")
    w2 = nc.dram_tensor("w2", [128, 1], F32, kind="ExternalInput")
    out = nc.dram_tensor("out", [NL, O, H, W_DIM], F16, kind="ExternalOutput")

    with tile.TileContext(nc) as tc:
        with (
            tc.tile_pool(name="const", bufs=1) as cpool,
            tc.tile_pool(name="imgs", bufs=2) as ipool,
            tc.tile_pool(name="outs", bufs=4) as opool,
            tc.tile_pool(name="psum", bufs=8, space="PSUM") as ppool,
        ):
            # PE warmup: memset scratch, then NWARM dummy matmuls so the
            # tensor engine's DVFS ramp happens during the DMA fill.
            wlhs = cpool.tile([128, 128], BF16)
            nc.gpsimd.memset(wlhs[:], 0.0)
            wrhs = cpool.tile([128, RCH, W_DIM], BF16)
            nc.gpsimd.memset(wrhs[:], 0.0)
            wps = ppool.tile([128, RCH, W_DIM], F32, tag="ps", name="wps")
            for i in range(NWARM):
                nc.tensor.matmul(
                    wps[:],
                    wlhs[:],
                    wrhs[:],
                    start=(i == 0),
                    stop=(i == NWARM - 1),
                )

            # Weights on the Scalar HWDGE queue: slot-0 taps split out so
            # the first matmul is gated on a small load.
            lwb0 = cpool.tile([128, 1, 128], BF16)
            nc.scalar.dma_start(out=lwb0[:], in_=lwb[:, 0:1, :])
            lwbr = cpool.tile([128, NXTAP - 1, 128], BF16)
            nc.scalar.dma_start(out=lwbr[:], in_=lwb[:, 1:NXTAP, :])
            lwot = cpool.tile([128, 128], BF16)
            nc.scalar.dma_start(out=lwot[:], in_=lwo[:, :])
            w2t = cpool.tile([128, 1], F32)
            nc.scalar.dma_start(out=w2t[:], in_=w2[:, :])

            def tap_w(slot):
                return lwb0[:, 0, :] if slot == 0 else lwbr[:, slot - 1, :]

            # Input DMAs up-front on SP, in consumption order.  Both
            # images of a pair are consecutive in DRAM: one DMA per group.
            xtiles = {}
            for p in range(NPAIR):
                na = 2 * p
                for gi, (r0, R, _chs) in enumerate(GROUPS):
                    xbh = ipool.tile(
                        [128, R, WP], BF16, tag=f"xbh{gi}", name=f"xbh_{p}_{gi}"
                    )
                    nc.sync.dma_start(
                        out=xbh[:, :, :], in_=xsb[na : na + 2, :, r0 : r0 + R, :]
                    )
                    xtiles[(p, gi)] = xbh

            ots = {}

            def finish(item):
                ch, na, nb, psa, psb, bh, r0, gi, last_in_group = item
                lb = ch * RCH - r0
                for half, ps in ((slice(0, 64), psa), (slice(64, 128), psb)):
                    nc.tensor.matmul(
                        ps[:],
                        lwot[half, :],
                        bh[half, lb : lb + RCH, :],
                        start=False,
                        stop=True,
                    )
                grows = RCH * len(GROUPS[gi][2])
                row = (ch * RCH - r0) % grows
                if row == 0:
                    for n_img in (na, nb):
                        ots[n_img] = opool.tile(
                            [128, grows, W_DIM], F16, tag="ot", name=f"ot{n_img}_{gi}"
                        )
                for ps, n_img in ((psa, na), (psb, nb)):
                    ot = ots[n_img]
                    nc.scalar.activation(
                        out=ot[:, row : row + RCH, :],
                        in_=ps[:],
                        func=mybir.ActivationFunctionType.Sqrt,
                        bias=w2t[:],
                        scale=1.0,
                    )
                    if last_in_group:
                        nc.sync.dma_start(
                            out=out[n_img, :, r0 : r0 + grows, :],
                            in_=ot[:, :, :],
                        )

            pending = []
            for p in range(NPAIR):
                na, nb = 2 * p, 2 * p + 1
                for gi, (r0, R, chs) in enumerate(GROUPS):
                    xbh = xtiles[(p, gi)]
                    # bf16 box pipeline on DVE: sq = x*x, separable 3x3 sum
                    sq = ipool.tile([128, R, WP], BF16, tag=f"sq{gi}", name=f"sq{gi}")
                    nc.vector.tensor_mul(sq[:], xbh[:], xbh[:])
                    uh = ipool.tile([128, R, W_DIM], BF16, tag=f"uh{gi}", name=f"uh{gi}")
                    nc.vector.tensor_add(uh[:], sq[:, :, 0:56], sq[:, :, 1:57])
                    tth = ipool.tile(
                        [128, R, W_DIM], BF16, tag=f"tth{gi}", name=f"tth{gi}"
                    )
                    nc.vector.tensor_add(tth[:], uh[:], sq[:, :, 2:58])
                    vh = ipool.tile(
                        [128, R - 2, W_DIM], BF16, tag=f"vh{gi}", name=f"vh{gi}"
                    )
                    nc.vector.tensor_add(
                        vh[:], tth[:, 0 : R - 2, :], tth[:, 1 : R - 1, :]
                    )
                    bh = ipool.tile(
                        [128, R - 2, W_DIM], BF16, tag=f"bh{gi}", name=f"bh{gi}"
                    )
                    nc.vector.tensor_add(bh[:], vh[:], tth[:, 2:R, :])

                    for ci, ch in enumerate(chs):
                        lh = ch * RCH - r0  # chunk's first row, local to group
                        psa = ppool.tile([128, RCH, W_DIM], F32, tag="ps", name="psa")
                        psb = ppool.tile([128, RCH, W_DIM], F32, tag="ps", name="psb")
                        for slot in range(NXTAP):
                            kh, kw = divmod(slot, 3)
                            rhs = xbh[:, lh + kh : lh + kh + RCH, kw : kw + 56]
                            st = slot == 0
                            lw = tap_w(slot)
                            nc.tensor.matmul(
                                psa[:],
                                lw[0:64],
                                rhs[0:64],
                                start=st,
                                stop=False,
                            )
                            nc.tensor.matmul(
                                psb[:],
                                lw[64:128],
                                rhs[64:128],
                                start=st,
                                stop=False,
                            )
                        pending.append(
                            (ch, na, nb, psa, psb, bh, r0, gi, ci == len(chs) - 1)
                        )
                        if len(pending) > DELAY:
                            finish(pending.pop(0))
            for item in pending:
                finish(item)
    nc.compile()
    return nc


def _host_weights(W):
    """bf16 x-tap lhsT [128, 9, 128] (dup on both halves), bf16 ones, w2."""
    W = np.asarray(W, np.float32)
    lhs = np.zeros((128, NXTAP, 128), np.float32)
    cidx = np.arange(C)
    for kh in range(3):
        for kw in range(3):
            slot = kh * 3 + kw
            blk = (-2.0 * W[:, cidx * 9 + kh * 3 + kw]).T  # [C, O]
            lhs[0:64, slot, :] = blk
            lhs[64:128, slot, :] = blk
    lwo = np.ones((128, 128), np.float32)
    w2 = (W * W).sum(axis=1).astype(np.float32).reshape(128, 1)
    return (
        lhs.astype(ml_dtypes.bfloat16),
        lwo.astype(ml_dtypes.bfloat16),
        w2,
    )


def get_program():
    global _PROGRAM
    if _PROGRAM is None:
        _PROGRAM = _build_program()
    return _PROGRAM


def make_in_maps(x, W):
    x = np.asarray(x, np.float32)
    xpad = np.zeros((N, C, HP, WP), np.float32)
    xpad[:, :, 1 : H + 1, 1 : W_DIM + 1] = x
    xpadb = np.ascontiguousarray(xpad.astype(ml_dtypes.bfloat16))
    lwb, lwo, w2 = _host_weights(W)
    return [
        {
            "xsb": xpadb[i * NL : (i + 1) * NL],
            "lwb": lwb,
            "lwo": lwo,
            "w2": w2,
        }
        for i in range(NCORES)
    ]


def kernel(x, W):
    nc = get_program()
    in_maps = make_in_maps(x, W)
    res = run_bass_kernel_spmd(nc, in_maps, list(range(NCORES)))
    outs = [res.results[i]["out"] for i in range(NCORES)]
    return np.concatenate(outs, axis=0).astype(np.float32)


# revision 9
# speedup vs baseline: 1.2884x; 1.0300x over previous
"""Trainium2 Bass kernel for Conv2D_DT (distance-transform conv).

d(n,o,h,w) = || patch(n,:,h,w) - W[o,:] ||_2  with 3x3/pad1 im2col patches.

Strategy (8 NeuronCores, data-parallel over batch):
  - 4 images per core, processed as 2 pairs: image A on SBUF partitions
    0-63, image B on partitions 64-127 (channels = partition dim).
  - d2 = ||p||^2 + ||w||^2 - 2 p.w  accumulated fully in PSUM:
      * 9 shifted matmuls (taps) with lhsT = -2*W_tap, bf16 [K=64/image]
      * 1 matmul with lhsT = ones (bf16) over b = 3x3 box sum of x^2,
        which is the whole ||p||^2 term (channel sum via the contraction)
  - The two images' K=64 matmuls land on PE row-groups (0,0)/(64,0) and
    run concurrently -> full 128-row array utilization (~78 TF/s).
  - All preprocessing in bf16 on DVE: sq = x*x (tensor_mul), 4 shifted
    adds for the separable 3x3 box sum.  Only the bf16 input is DMA'd;
    output is written as fp16 and upcast on host.
  - Startup: PE-warmup matmuls over memset scratch pay the DVFS ramp
    during the DMA-fill window; the slot-0 tap weights are a separate
    small DMA so the first real matmul is gated on a small load.  Both
    images of a group load with one DMA (consecutive in DRAM).
  - epilogue: ScalarE  out = Sqrt(psum + w2[o]) -> fp16, one output DMA
    per row-group.  (d2 >= ~200 for this data, Sqrt is safe.)
"""

import sys

_REPO = "/opt/trn_rl_repo"
if _REPO not in sys.path:
    sys.path.insert(0, _REPO)

import ml_dtypes
import numpy as np

import concourse.bass as bass  # noqa: F401
import concourse.mybir as mybir
import concourse.tile as tile
from concourse import bacc
from concourse.bass_utils import run_bass_kernel_spmd

# Problem geometry (hardcoded per harness contract).
N, C, H, W_DIM, O = 32, 64, 56, 56, 128
NCORES = 8
NL = N // NCORES  # images per core
NPAIR = NL // 2  # image pairs per core
HP = WP = 58  # zero-padded spatial dims
RCH = 8  # output rows per PSUM chunk
NCH = H // RCH  # 7 chunks per image
NXTAP = 9
DELAY = 1  # chunks between taps and box-matmul/epilogue
NWARM = 12  # PE-warmup matmuls to pay the DVFS ramp during DMA fill

F32 = mybir.dt.float32
F16 = mybir.dt.float16
BF16 = mybir.dt.bfloat16

# (r0, R, chunks): padded-row window [r0, r0+R) covering output chunks
GROUPS = ((0, 10, (0,)), (8, 18, (1, 2)), (24, 18, (3, 4)), (40, 18, (5, 6)))

_PROGRAM = None


def _build_program():
    nc = bacc.Bacc(
        "TRN2",
        target_bir_lowering=False,
        debug=False,
        enable_asserts=False,
        num_devices=NCORES,
    )
    xsb = nc.dram_tensor("xsb", [NL, C, HP, WP], BF16, kind="ExternalInput")
    lwb = nc.dram_tensor("lwb", [128, NXTAP, 128], BF16, kind="ExternalInput")
    lwo = nc.dram_tensor("lwo", [128, 128], BF16, kind="ExternalInput")
    w2 = nc.dram_tensor("w2", [128, 1], F32, kind="ExternalInput")
    out = nc.dram_tensor("out", [NL, O, H, W_DIM], F16, kind="ExternalOutput")

    with tile.TileContext(nc) as tc:
        with (
            tc.tile_pool(name="const", bufs=1) as cpool,
            tc.tile_pool(name="imgs", bufs=2) as ipool,
            tc.tile_pool(name="outs", bufs=4) as opool,
            tc.tile_pool(name="psum", bufs=8, space="PSUM") as ppool,
        ):
            # PE warmup: memset scratch, then NWARM dummy matmuls so the
            # tensor engine's DVFS ramp happens during the DMA fill.
            wlhs = cpool.tile([128, 128], BF16)
            nc.gpsimd.memset(wlhs[:], 0.0)
            wrhs = cpool.tile([128, RCH, W_DIM], BF16)
            nc.gpsimd.memset(wrhs[:], 0.0)
            wps = ppool.tile([128, RCH, W_DIM], F32, tag="ps", name="wps")
            for i in range(NWARM):
                nc.tensor.matmul(
                    wps[:],
                    wlhs[:],
                    wrhs[:],
                    start=(i == 0),
                    stop=(i == NWARM - 1),
                )

            # First group of pair 0: separate A/B loads (deps are
            # AP-granular, so the first taps gate only on the A half) with
            # the B half on the Scalar HWDGE queue, in parallel with SP.
            r0_0, R_0, _ = GROUPS[0]
            xbh0 = ipool.tile([128, R_0, WP], BF16, tag="xbh0", name="xbh0")
            nc.sync.dma_start(out=xbh0[0:64, :, :], in_=xsb[0, :, r0_0 : r0_0 + R_0, :])
            nc.scalar.dma_start(
                out=xbh0[64:128, :, :], in_=xsb[1, :, r0_0 : r0_0 + R_0, :]
            )
            # Weights on the Scalar HWDGE queue: slot-0 taps split out so
            # the first matmul is gated on a small load.
            lwb0 = cpool.tile([128, 1, 128], BF16)
            nc.scalar.dma_start(out=lwb0[:], in_=lwb[:, 0:1, :])
            lwbr = cpool.tile([128, NXTAP - 1, 128], BF16)
            nc.scalar.dma_start(out=lwbr[:], in_=lwb[:, 1:NXTAP, :])
            lwot = cpool.tile([128, 128], BF16)
            nc.scalar.dma_start(out=lwot[:], in_=lwo[:, :])
            w2t = cpool.tile([128, 1], F32)
            nc.scalar.dma_start(out=w2t[:], in_=w2[:, :])

            def tap_w(slot):
                return lwb0[:, 0, :] if slot == 0 else lwbr[:, slot - 1, :]

            # Remaining input DMAs up-front on SP, in consumption order.
            # Both images of a pair are consecutive in DRAM: one DMA/group.
            xtiles = {(0, 0): xbh0}
            for p in range(NPAIR):
                na = 2 * p
                for gi, (r0, R, _chs) in enumerate(GROUPS):
                    if (p, gi) in xtiles:
                        continue
                    xbh = ipool.tile(
                        [128, R, WP], BF16, tag=f"xbh{gi}", name=f"xbh_{p}_{gi}"
                    )
                    nc.sync.dma_start(
                        out=xbh[:, :, :], in_=xsb[na : na + 2, :, r0 : r0 + R, :]
                    )
                    xtiles[(p, gi)] = xbh

            ots = {}

            def finish(item):
                ch, na, nb, psa, psb, bh, r0, gi, last_in_group, split_out = item
                lb = ch * RCH - r0
                for half, ps in ((slice(0, 64), psa), (slice(64, 128), psb)):
                    nc.tensor.matmul(
                        ps[:],
                        lwot[half, :],
                        bh[half, lb : lb + RCH, :],
                        start=False,
                        stop=True,
                    )
                # Per-chunk output DMA on the drain path (split_out) so the
                # final chunk's epilogue doesn't wait on its group sibling;
                # otherwise one batched DMA per row-group.
                grows = RCH if split_out else RCH * len(GROUPS[gi][2])
                row = (ch * RCH - r0) % grows
                h0 = ch * RCH if split_out else r0
                if row == 0:
                    for n_img in (na, nb):
                        ots[n_img] = opool.tile(
                            [128, grows, W_DIM],
                            F16,
                            tag="ot",
                            name=f"ot{n_img}_{gi}_{ch}",
                        )
                for ps, n_img in ((psa, na), (psb, nb)):
                    ot = ots[n_img]
                    nc.scalar.activation(
                        out=ot[:, row : row + RCH, :],
                        in_=ps[:],
                        func=mybir.ActivationFunctionType.Sqrt,
                        bias=w2t[:],
                        scale=1.0,
                    )
                    if last_in_group or split_out:
                        nc.sync.dma_start(
                            out=out[n_img, :, h0 : h0 + grows, :],
                            in_=ot[:, :, :],
                        )

            pending = []
            for p in range(NPAIR):
                na, nb = 2 * p, 2 * p + 1
                for gi, (r0, R, chs) in enumerate(GROUPS):
                    xbh = xtiles[(p, gi)]
                    # bf16 box pipeline on DVE: sq = x*x, separable 3x3 sum
                    sq = ipool.tile([128, R, WP], BF16, tag=f"sq{gi}", name=f"sq{gi}")
                    nc.vector.tensor_mul(sq[:], xbh[:], xbh[:])
                    uh = ipool.tile([128, R, W_DIM], BF16, tag=f"uh{gi}", name=f"uh{gi}")
                    nc.vector.tensor_add(uh[:], sq[:, :, 0:56], sq[:, :, 1:57])
                    tth = ipool.tile(
                        [128, R, W_DIM], BF16, tag=f"tth{gi}", name=f"tth{gi}"
                    )
                    nc.vector.tensor_add(tth[:], uh[:], sq[:, :, 2:58])
                    vh = ipool.tile(
                        [128, R - 2, W_DIM], BF16, tag=f"vh{gi}", name=f"vh{gi}"
                    )
                    nc.vector.tensor_add(
                        vh[:], tth[:, 0 : R - 2, :], tth[:, 1 : R - 1, :]
                    )
                    bh = ipool.tile(
                        [128, R - 2, W_DIM], BF16, tag=f"bh{gi}", name=f"bh{gi}"
                    )
                    nc.vector.tensor_add(bh[:], vh[:], tth[:, 2:R, :])

                    for ci, ch in enumerate(chs):
                        lh = ch * RCH - r0  # chunk's first row, local to group
                        psa = ppool.tile([128, RCH, W_DIM], F32, tag="ps", name="psa")
                        psb = ppool.tile([128, RCH, W_DIM], F32, tag="ps", name="psb")
                        for slot in range(NXTAP):
                            kh, kw = divmod(slot, 3)
                            rhs = xbh[:, lh + kh : lh + kh + RCH, kw : kw + 56]
                            st = slot == 0
                            lw = tap_w(slot)
                            nc.tensor.matmul(
                                psa[:],
                                lw[0:64],
                                rhs[0:64],
                                start=st,
                                stop=False,
                            )
                            nc.tensor.matmul(
                                psb[:],
                                lw[64:128],
                                rhs[64:128],
                                start=st,
                                stop=False,
                            )
                        split_out = p == NPAIR - 1 and gi == len(GROUPS) - 1
                        pending.append(
                            (
                                ch,
                                na,
                                nb,
                                psa,
                                psb,
                                bh,
                                r0,
                                gi,
                                ci == len(chs) - 1,
                                split_out,
                            )
                        )
                        if len(pending) > DELAY:
                            finish(pending.pop(0))
            for item in pending:
                finish(item)
    nc.compile()
    return nc


def _host_weights(W):
    """bf16 x-tap lhsT [128, 9, 128] (dup on both halves), bf16 ones, w2."""
    W = np.asarray(W, np.float32)
    lhs = np.zeros((128, NXTAP, 128), np.float32)
    cidx = np.arange(C)
    for kh in range(3):
        for kw in range(3):
            slot = kh * 3 + kw
            blk = (-2.0 * W[:, cidx * 9 + kh * 3 + kw]).T  # [C, O]
            lhs[0:64, slot, :] = blk
            lhs[64:128, slot, :] = blk
    lwo = np.ones((128, 128), np.float32)
    w2 = (W * W).sum(axis=1).astype(np.float32).reshape(128, 1)
    return (
        lhs.astype(ml_dtypes.bfloat16),
        lwo.astype(ml_dtypes.bfloat16),
        w2,
    )


def get_program():
    global _PROGRAM
    if _PROGRAM is None:
        _PROGRAM = _build_program()
    return _PROGRAM


def make_in_maps(x, W):
    x = np.asarray(x, np.float32)
    xpad = np.zeros((N, C, HP, WP), np.float32)
    xpad[:, :, 1 : H + 1, 1 : W_DIM + 1] = x
    xpadb = np.ascontiguousarray(xpad.astype(ml_dtypes.bfloat16))
    lwb, lwo, w2 = _host_weights(W)
    return [
        {
            "xsb": xpadb[i * NL : (i + 1) * NL],
            "lwb": lwb,
            "lwo": lwo,
            "w2": w2,
        }
        for i in range(NCORES)
    ]


def kernel(x, W):
    nc = get_program()
    in_maps = make_in_maps(x, W)
    res = run_bass_kernel_spmd(nc, in_maps, list(range(NCORES)))
    outs = [res.results[i]["out"] for i in range(NCORES)]
    return np.concatenate(outs, axis=0).astype(np.float32)
